# revision 1
# baseline (speedup 1.0000x reference)
"""Differential self-attention (B=2,T=2048,C=1024,H=16) on 8 trn2 NeuronCores.

Sharding: core c owns global heads {2c, 2c+1} for BOTH batches, and the
output shard (batch c//4, T-quarter c%4). Per core: column-parallel QKV
projections (fp32r matmuls), causal differential attention in S^T layout
(k on partitions, q on free; exp on ACT straight PSUM->SBUF; causal
masking via trapezoid-narrowed matmuls; softmax denominators via a
ones-column folded into the PV matmul; per-q normalization applied with
PE-replicated reciprocal rows), then an 8-core AllToAll redistributes
y^T head-shards into (batch, T-quarter) shards, and the core runs full
wo + compressor (wc/we) on its local T-quarter. Host only
slices/transposes inputs and concatenates outputs.
"""
import math
import sys

import numpy as np

for _p in ("/opt/trn_rl_repo", "/opt/trn_rl_repo/concourse"):
    if _p not in sys.path:
        sys.path.insert(0, _p)

import concourse.bass as bass  # noqa: E402
import concourse.tile as tile  # noqa: E402
from concourse import bacc, mybir  # noqa: E402
from concourse.bass_utils import run_bass_kernel_spmd  # noqa: E402

B, T, C, H = 2, 2048, 1024, 16
DH = C // H  # 64
N_LAYER = 12
LAMBDA_INIT = 0.8 - 0.6 * math.exp(-0.3 * (N_LAYER - 1))
SCALE = 1.0 / math.sqrt(DH)

TQ = 512        # q tile (free dim)
KBS = 128       # k block (partition dim)
NQT = T // TQ   # 4
NKB = T // KBS  # 16
TC = 512        # xT streaming chunk (T columns per chunk)
NTC = T // TC   # 4

F32 = mybir.dt.float32
F32R = mybir.dt.float32r
EXP = mybir.ActivationFunctionType.Exp

_CACHE = {}


def _build():
    nc = bacc.Bacc("TRN2", target_bir_lowering=False, debug=False, num_devices=8)
    d = nc.dram_tensor
    xT0 = d("xT0", [C, T], F32R, kind="ExternalInput").ap()
    xT1 = d("xT1", [C, T], F32R, kind="ExternalInput").ap()
    wqT = d("wqT", [C, 256], F32R, kind="ExternalInput").ap()
    wkT = d("wkT", [C, 256], F32R, kind="ExternalInput").ap()
    wvP = d("wvP", [C, 256], F32R, kind="ExternalInput").ap()  # cols 0-127 real
    woT = d("woT", [C, C], F32R, kind="ExternalInput").ap()
    wcT = d("wcT", [C, 512], F32R, kind="ExternalInput").ap()
    weT = d("weT", [512, C], F32R, kind="ExternalInput").ap()
    bcT = d("bcT", [128, 4], F32, kind="ExternalInput").ap()
    beR = d("beR", [128, C], F32, kind="ExternalInput").ap()
    lv = d("lv", [1, 64], F32, kind="ExternalInput").ap()
    mk = d("mk", [128, 128], F32, kind="ExternalInput").ap()
    out = d("out", [TQ, C], F32, kind="ExternalOutput").ap()

    r3 = lambda ap: ap.rearrange("(ko p) m -> p ko m", p=128)  # noqa: E731
    with tile.TileContext(nc) as tc:
        _emit(nc, tc, (r3(xT0), r3(xT1)), r3(wqT), r3(wkT), r3(wvP),
              r3(woT), r3(wcT), r3(weT), bcT, beR, lv, mk, out)
    nc.compile()
    return nc


def _emit(nc, tc, xT3, wqT3, wkT3, wvP3, woT3, wcT3, weT3, bcT, beR, lv, mk, out):
    from contextlib import ExitStack

    ctx = ExitStack()
    with ctx:
        const = ctx.enter_context(tc.tile_pool(name="const", bufs=1))
        tailw = ctx.enter_context(tc.tile_pool(name="tailw", bufs=1))
        attn_ctx = ctx.enter_context(ExitStack())
        qk = attn_ctx.enter_context(tc.tile_pool(name="qk", bufs=1))
        vpool = attn_ctx.enter_context(tc.tile_pool(name="vpool", bufs=1))
        dram = ctx.enter_context(tc.tile_pool(name="dram", bufs=1, space="DRAM"))

        # ---- lam = exp(sum(lq1*lk1)) - exp(sum(lq2*lk2)) + LAMBDA_INIT ----
        lv_sb = const.tile([1, 64], F32)
        nc.sync.dma_start(lv_sb[:], lv)
        ll = const.tile([1, 32], F32)
        nc.vector.tensor_mul(ll[:, 0:16], lv_sb[:, 0:16], lv_sb[:, 16:32])
        nc.vector.tensor_mul(ll[:, 16:32], lv_sb[:, 32:48], lv_sb[:, 48:64])
        ss = const.tile([1, 2], F32)
        nc.vector.reduce_sum(ss[:, 0:1], ll[:, None, 0:16], axis=mybir.AxisListType.X)
        nc.vector.reduce_sum(ss[:, 1:2], ll[:, None, 16:32], axis=mybir.AxisListType.X)
        es = const.tile([1, 2], F32)
        nc.scalar.activation(es[:], ss[:], EXP)  # loads exp table early too
        lam = const.tile([1, 1], F32)
        nc.vector.tensor_sub(lam[:], es[:, 0:1], es[:, 1:2])
        nc.vector.tensor_scalar_add(lam[:], lam[:], LAMBDA_INIT)
        # ones/lam rows to replicate r1 / lam*r2 onto 64 partitions via K=1 matmuls
        ones_f = const.tile([1, 64], F32)
        nc.vector.memset(ones_f[:], 1.0)
        onesr = const.tile([1, 64], F32R)
        nc.vector.tensor_copy(onesr[:], ones_f[:])
        lam_row = const.tile([1, 64], F32)
        nc.vector.tensor_copy(lam_row[:], lam[0:1, 0:1].to_broadcast([1, 64]))
        lamones = const.tile([1, 64], F32R)
        nc.vector.tensor_copy(lamones[:], lam_row[:])
        mk_f = const.tile([128, 128], F32)
        nc.sync.dma_start(mk_f[:], mk)
        mk_r = const.tile([128, 128], F32R)
        nc.vector.tensor_copy(mk_r[:], mk_f[:])
        be_sb = const.tile([128, C], F32)
        bc_sb = const.tile([128, 4], F32)
        nc.sync.dma_start(be_sb[:], beR)
        nc.sync.dma_start(bc_sb[:], bcT)

        # ---- persistent big tiles ----
        BF16 = mybir.dt.bfloat16
        qT1 = qk.tile([128, 2, T], BF16)   # [2heads x 64, batch, T]
        qT2 = qk.tile([128, 2, T], BF16)
        kT1 = qk.tile([128, 2, T], BF16)
        kT2 = qk.tile([128, 2, T], BF16)
        v5 = vpool.tile([128, NKB, 2, 2, 65], F32R)  # [Tmod, Tdiv, batch, head, dh+1]
        ones128 = const.tile([128, 1], F32)
        nc.vector.memset(ones128[:], 1.0)
        nc.vector.tensor_copy(
            v5[:, :, :, :, 64:65],
            ones128[:, 0:1, None, None, None].to_broadcast([128, NKB, 2, 2, 1]),
        )
        a2a_in = [dram.tile([8, 64, TQ], F32R, name=f"a2ain{h}") for h in range(2)]
        a2a_out = [dram.tile([8, 64, TQ], F32R, name=f"a2aout{h}") for h in range(2)]

        # ---- phase 1: QKV projections, streaming xT chunks ----
        with tc.tile_pool(name="projw", bufs=1) as projw, \
             tc.tile_pool(name="xs", bufs=2) as xs, \
             tc.tile_pool(name="pproj", bufs=3, space="PSUM") as pproj:
            wq_sb = projw.tile([128, 8, 256], F32R)
            wk_sb = projw.tile([128, 8, 256], F32R)
            wv_sb = projw.tile([128, 8, 256], F32R)
            nc.gpsimd.dma_start(wq_sb[:], wqT3)
            nc.gpsimd.dma_start(wk_sb[:], wkT3)
            nc.gpsimd.dma_start(wv_sb[:], wvP3)
            for b2 in range(2):
                for tcn in range(NTC):
                    xc = xs.tile([128, 8, TC], F32R, tag="xc", name="xc")
                    nc.sync.dma_start(xc[:], xT3[b2][:, :, bass.ts(tcn, TC)])
                    for w_sb, dst1, dst2 in ((wq_sb, qT1, qT2), (wk_sb, kT1, kT2)):
                        for m in range(2):
                            ps = pproj.tile([128, TC], F32, tag="pqk", name="ps_qk")
                            for k in range(8):
                                nc.tensor.matmul(ps[:], w_sb[:, k, bass.ts(m, 128)],
                                                 xc[:, k, :], start=(k == 0), stop=(k == 7))
                            dst = dst1 if m == 0 else dst2
                            nc.vector.tensor_copy(dst[:, b2, bass.ts(tcn, TC)], ps[:])
                    for tt in range(TC // 128):
                        psv = pproj.tile([128, 256], F32, tag="pv", name="ps_v")
                        for k in range(8):
                            nc.tensor.matmul(psv[:], xc[:, k, bass.ts(tt, 128)],
                                             wv_sb[:, k, :], start=(k == 0), stop=(k == 7))
                        nc.vector.tensor_copy(
                            v5[:, tcn * 4 + tt, b2, :, 0:64],
                            psv[:, 0:128].rearrange("p (h d) -> p h d", h=2),
                        )

        # ---- phase 2: attention (wo prefetches meanwhile) ----
        wo_sb = tailw.tile([128, 8, C], F32R)
        nc.sync.dma_start(wo_sb[:], woT3)
        ypool = attn_ctx.enter_context(tc.tile_pool(name="ypool", bufs=1))
        yT = ypool.tile([128, 2, T], F32R)  # [2heads x 64, batch, T]

        with tc.tile_pool(name="epool", bufs=2) as epool, \
             tc.tile_pool(name="cmb", bufs=2) as cmb, \
             tc.tile_pool(name="psS", bufs=2, space="PSUM") as psS, \
             tc.tile_pool(name="psU", bufs=1, space="PSUM") as psU, \
             tc.tile_pool(name="psR", bufs=1, space="PSUM") as psR:
            for h2 in range(2):
                hb = 64 * h2
                for b2 in range(2):
                    for qt in range(NQT):
                        kmax = 4 * qt + 4
                        us = []
                        for a, (qTa, kTa) in enumerate(((qT1, kT1), (qT2, kT2))):
                            u = psU.tile([65, TQ], F32, tag=f"U{a}", name=f"u{a}")
                            for kb0 in range(0, kmax, 2):
                                st = psS.tile([128, 2, TQ], F32, tag="S", name="st")
                                e = epool.tile([128, 2, TQ], F32R, tag="E", name="et")
                                offs = []
                                for i in (0, 1):
                                    kb = kb0 + i
                                    qo = max(0, (kb - 4 * qt) * 128)
                                    offs.append(qo)
                                    nc.tensor.matmul(
                                        st[:, i, qo:],
                                        kTa[hb:hb + 64, b2, bass.ts(kb, KBS)],
                                        qTa[hb:hb + 64, b2, qt * TQ + qo:(qt + 1) * TQ],
                                        start=True, stop=True)
                                mo = min(offs)
                                nc.scalar.activation(e[:, :, mo:], st[:, :, mo:],
                                                     EXP, scale=SCALE)
                                for i in (0, 1):
                                    kb, qo = kb0 + i, offs[i]
                                    if kb >= 4 * qt:  # diagonal: staircase mask
                                        nc.vector.tensor_mul(e[:, i, qo:qo + 128],
                                                             e[:, i, qo:qo + 128],
                                                             mk_r[:])
                                    nc.tensor.matmul(
                                        u[:, qo:], v5[:, kb, b2, h2, :], e[:, i, qo:],
                                        start=(kb == 0), stop=(kb == kmax - 1),
                                        skip_group_check=True)
                            us.append(u)
                        # combine: yT[:, qt] = U1/Z1 - lam*U2/Z2
                        zr1 = cmb.tile([1, TQ], F32, tag="zr1", name="zr1")
                        zr2 = cmb.tile([1, TQ], F32, tag="zr2", name="zr2")
                        nc.scalar.copy(zr1[:], us[0][64:65, :])
                        nc.scalar.copy(zr2[:], us[1][64:65, :])
                        rzf = cmb.tile([1, 2 * TQ], F32, tag="rzf", name="rzf")
                        nc.vector.reciprocal_approx_fast(rzf[:, 0:TQ], zr1[:])
                        nc.vector.reciprocal_approx_fast(rzf[:, TQ:], zr2[:])
                        rz = cmb.tile([1, 2 * TQ], F32R, tag="rz", name="rz")
                        nc.vector.tensor_copy(rz[:], rzf[:])
                        rb1 = psR.tile([64, TQ], F32, tag="rb1", name="rb1")
                        rb2 = psR.tile([64, TQ], F32, tag="rb2", name="rb2")
                        nc.tensor.matmul(rb1[:], onesr[:], rz[:, 0:TQ], start=True, stop=True)
                        nc.tensor.matmul(rb2[:], lamones[:], rz[:, TQ:], start=True, stop=True)
                        rbs = cmb.tile([128, TQ], F32, tag="rbs", name="rbs")
                        nc.scalar.copy(rbs[0:64, :], rb1[:])
                        nc.scalar.copy(rbs[64:128, :], rb2[:])
                        t1 = cmb.tile([64, TQ], F32, tag="t1", name="t1")
                        t2 = cmb.tile([64, TQ], F32, tag="t2", name="t2")
                        nc.vector.tensor_mul(t1[:], us[0][0:64, :], rbs[0:64, :])
                        nc.vector.tensor_mul(t2[:], us[1][0:64, :], rbs[64:128, :])
                        nc.vector.tensor_sub(yT[hb:hb + 64, b2, bass.ts(qt, TQ)],
                                             t1[:], t2[:])
                    # ship (h2, b2) yT rows into the A2A input shards
                    nc.sync.dma_start(
                        a2a_in[h2][b2 * 4:(b2 + 1) * 4, :, :].rearrange("j p t -> p j t"),
                        yT[hb:hb + 64, b2, :].rearrange("p (j t) -> p j t", t=TQ),
                    )
                nc.gpsimd.collective_compute(
                    "AllToAll", mybir.AluOpType.bypass,
                    replica_groups=[list(range(8))],
                    ins=[a2a_in[h2].opt()], outs=[a2a_out[h2].opt()],
                )

        # ---- phase 3: wo + compressor on the local (batch, T-quarter) ----
        attn_ctx.close()
        with tc.tile_pool(name="tail", bufs=1) as tail, \
             tc.tile_pool(name="opool", bufs=2) as opool, \
             tc.tile_pool(name="psT", bufs=2, space="PSUM") as psT:
            wc_sb = tail.tile([128, 8, 512], F32R)
            we_sb = tail.tile([128, 4, C], F32R)
            nc.sync.dma_start(wc_sb[:], wcT3)
            nc.sync.dma_start(we_sb[:], weT3)
            yf = tail.tile([128, 8, TQ], F32R)
            for kc in range(8):
                for h2 in range(2):
                    nc.sync.dma_start(yf[h2 * 64:(h2 + 1) * 64, kc, :],
                                      a2a_out[h2][kc, :, :])
            zT = tail.tile([128, 8, TQ], F32R)
            for mt in range(8):
                pz = psT.tile([128, TQ], F32, tag="pz", name="pz")
                for kc in range(8):
                    nc.tensor.matmul(pz[:], wo_sb[:, kc, bass.ts(mt, 128)], yf[:, kc, :],
                                     start=(kc == 0), stop=(kc == 7))
                nc.vector.tensor_copy(zT[:, mt, :], pz[:])
            hT = tail.tile([128, 4, TQ], F32R)
            for mt in range(4):
                ph = psT.tile([128, TQ], F32, tag="ph", name="ph")
                for kc in range(8):
                    nc.tensor.matmul(ph[:], wc_sb[:, kc, bass.ts(mt, 128)], zT[:, kc, :],
                                     start=(kc == 0), stop=(kc == 7))
                nc.scalar.add(hT[:, mt, :], ph[:], bc_sb[:, mt:mt + 1])
            for tt in range(4):
                o = opool.tile([128, C], F32, tag="o", name="o")
                for half in range(2):
                    po = psT.tile([128, TQ], F32, tag="po", name="po")
                    for kc in range(4):
                        nc.tensor.matmul(po[:], hT[:, kc, bass.ts(tt, 128)],
                                         we_sb[:, kc, bass.ts(half, TQ)],
                                         start=(kc == 0), stop=(kc == 3))
                    nc.vector.tensor_add(o[:, bass.ts(half, TQ)], po[:],
                                         be_sb[:, bass.ts(half, TQ)])
                nc.sync.dma_start(out[bass.ts(tt, 128), :], o[:])


def _prep_inputs(inputs):
    g = {k: np.asarray(v, dtype=np.float32) for k, v in inputs.items()}
    x, wq, wk, wv, wo = g["x"], g["wq"], g["wk"], g["wv"], g["wo"]
    wc, bc, we, be = g["wc"], g["bc"], g["we"], g["be"]
    lv = np.concatenate([g["lq1"], g["lk1"], g["lq2"], g["lk2"]]).reshape(1, 64).astype(np.float32)
    mk = np.ascontiguousarray(np.tril(np.ones((128, 128), np.float32)).T)
    woT = np.ascontiguousarray(wo.T)
    wcT = np.ascontiguousarray(wc.T)
    weT = np.ascontiguousarray(we.T)
    bcT = np.ascontiguousarray(bc.reshape(4, 128).T)
    beR = np.ascontiguousarray(np.broadcast_to(be[None, :], (128, C)))
    xT0 = np.ascontiguousarray(x[0].T)
    xT1 = np.ascontiguousarray(x[1].T)
    in_maps = []
    for c in range(8):
        r0, r1 = c * 128, (c + 1) * 128
        wqs = np.ascontiguousarray(np.concatenate([wq[r0:r1], wq[C + r0:C + r1]], 0).T)
        wks = np.ascontiguousarray(np.concatenate([wk[r0:r1], wk[C + r0:C + r1]], 0).T)
        wvs = np.zeros((C, 256), dtype=np.float32)
        wvs[:, 0:128] = wv[r0:r1].T
        in_maps.append({
            "xT0": xT0, "xT1": xT1, "wqT": wqs, "wkT": wks,
            "wvP": np.ascontiguousarray(wvs), "woT": woT,
            "wcT": wcT, "weT": weT, "bcT": bcT, "beR": beR, "lv": lv, "mk": mk,
        })
    return in_maps


def _run(inputs, trace=False, trace_cores=None):
    if "nc" not in _CACHE:
        _CACHE["nc"] = _build()
    in_maps = _prep_inputs(inputs)
    r = run_bass_kernel_spmd(
        _CACHE["nc"], in_maps, core_ids=list(range(8)), trace=trace,
        trace_cores=trace_cores,
    )
    o = np.empty((B, T, C), dtype=np.float32)
    for c in range(8):
        b, hg = c // 4, c % 4
        o[b, hg * TQ:(hg + 1) * TQ, :] = r.results[c]["out"]
    return o, r


def kernel(**inputs) -> np.ndarray:
    o, _ = _run(inputs, trace=False)
    return o



# revision 12
# speedup vs baseline: 1.3945x; 1.3945x over previous
"""Differential self-attention (B=2,T=2048,C=1024,H=16) on 8 trn2 NeuronCores.

Sharding: core c owns global heads {2c, 2c+1} for BOTH batches, and the
output shard (batch c//4, T-quarter c%4). Per core: column-parallel QKV
projections (fp32r matmuls), causal differential attention in S^T layout
(k on partitions, q on free; exp on ACT straight PSUM->SBUF in bf16;
causal masking via trapezoid-narrowed matmuls; softmax denominators via
a ones-column folded into the PV matmul). The two branches' S->exp->PV
chains are software-pipelined (interleaved, PV deferred 2 steps) so the
PE never stalls on the ACT exp latency. lam is pre-folded into a second
bf16 V copy; per-q normalization uses reciprocal rows broadcast across
partitions on GPSIMD. An 8-core AllToAll (bf16) redistributes y^T
head-shards into (batch, T-quarter) shards, and the core runs full
wo + compressor (wc/we) on its local T-quarter. Host only
slices/transposes inputs and concatenates outputs.
"""
import math
import sys

import numpy as np

for _p in ("/opt/trn_rl_repo", "/opt/trn_rl_repo/concourse"):
    if _p not in sys.path:
        sys.path.insert(0, _p)

import concourse.bass as bass  # noqa: E402
import concourse.tile as tile  # noqa: E402
from concourse import bacc, mybir  # noqa: E402
from concourse.bass_utils import run_bass_kernel_spmd  # noqa: E402

B, T, C, H = 2, 2048, 1024, 16
DH = C // H  # 64
N_LAYER = 12
LAMBDA_INIT = 0.8 - 0.6 * math.exp(-0.3 * (N_LAYER - 1))
SCALE = 1.0 / math.sqrt(DH)

TQ = 512        # q tile (free dim)
KBS = 128       # k block (partition dim)
NQT = T // TQ   # 4
NKB = T // KBS  # 16
TC = 512        # xT streaming chunk (T columns per chunk)
NTC = T // TC   # 4

F32 = mybir.dt.float32
F32R = mybir.dt.float32r
BF16 = mybir.dt.bfloat16
EXP = mybir.ActivationFunctionType.Exp

_CACHE = {}


def _build():
    nc = bacc.Bacc("TRN2", target_bir_lowering=False, debug=False, num_devices=8)
    d = nc.dram_tensor
    xT0 = d("xT0", [C, T], F32R, kind="ExternalInput").ap()
    xT1 = d("xT1", [C, T], F32R, kind="ExternalInput").ap()
    wqT = d("wqT", [C, 256], F32R, kind="ExternalInput").ap()
    wkT = d("wkT", [C, 256], F32R, kind="ExternalInput").ap()
    wvP = d("wvP", [C, 256], F32R, kind="ExternalInput").ap()  # cols 0-127 real
    woT = d("woT", [C, C], F32R, kind="ExternalInput").ap()
    wcT = d("wcT", [C, 512], F32R, kind="ExternalInput").ap()
    weT = d("weT", [512, C], F32R, kind="ExternalInput").ap()
    bcT = d("bcT", [128, 4], F32, kind="ExternalInput").ap()
    beR = d("beR", [128, C], F32, kind="ExternalInput").ap()
    lv = d("lv", [1, 64], F32, kind="ExternalInput").ap()
    mk = d("mk", [128, 128], F32, kind="ExternalInput").ap()
    out = d("out", [TQ, C], F32, kind="ExternalOutput").ap()

    r3 = lambda ap: ap.rearrange("(ko p) m -> p ko m", p=128)  # noqa: E731
    with tile.TileContext(nc) as tc:
        _emit(nc, tc, (r3(xT0), r3(xT1)), r3(wqT), r3(wkT), r3(wvP),
              r3(woT), r3(wcT), r3(weT), bcT, beR, lv, mk, out)
    nc.compile()
    return nc


def _emit(nc, tc, xT3, wqT3, wkT3, wvP3, woT3, wcT3, weT3, bcT, beR, lv, mk, out):
    from contextlib import ExitStack

    ctx = ExitStack()
    with ctx:
        const = ctx.enter_context(tc.tile_pool(name="const", bufs=1))
        tailw = ctx.enter_context(tc.tile_pool(name="tailw", bufs=1))
        attn_ctx = ctx.enter_context(ExitStack())
        qk = attn_ctx.enter_context(tc.tile_pool(name="qk", bufs=1))
        vpool = attn_ctx.enter_context(tc.tile_pool(name="vpool", bufs=1))
        dram = ctx.enter_context(tc.tile_pool(name="dram", bufs=1, space="DRAM"))

        # ---- lam = exp(sum(lq1*lk1)) - exp(sum(lq2*lk2)) + LAMBDA_INIT ----
        lv_sb = const.tile([1, 64], F32)
        nc.sync.dma_start(lv_sb[:], lv)
        ll = const.tile([1, 32], F32)
        nc.vector.tensor_mul(ll[:, 0:16], lv_sb[:, 0:16], lv_sb[:, 16:32])
        nc.vector.tensor_mul(ll[:, 16:32], lv_sb[:, 32:48], lv_sb[:, 48:64])
        ss = const.tile([1, 2], F32)
        nc.vector.reduce_sum(ss[:, 0:1], ll[:, None, 0:16], axis=mybir.AxisListType.X)
        nc.vector.reduce_sum(ss[:, 1:2], ll[:, None, 16:32], axis=mybir.AxisListType.X)
        es = const.tile([1, 2], F32)
        nc.scalar.activation(es[:], ss[:], EXP)  # loads exp table early too
        lam = const.tile([1, 1], F32)
        nc.vector.tensor_sub(lam[:], es[:, 0:1], es[:, 1:2])
        nc.vector.tensor_scalar_add(lam[:], lam[:], LAMBDA_INIT)
        lam128 = const.tile([128, 1], F32)
        nc.gpsimd.partition_broadcast(lam128[:], lam[:])
        mk_f = const.tile([128, 128], F32)
        nc.sync.dma_start(mk_f[:], mk)
        mk_b = const.tile([128, 128], BF16)
        nc.vector.tensor_copy(mk_b[:], mk_f[:])
        be_sb = const.tile([128, C], F32)
        bc_sb = const.tile([128, 4], F32)
        nc.sync.dma_start(be_sb[:], beR)
        nc.sync.dma_start(bc_sb[:], bcT)

        # ---- persistent big tiles ----
        qT1 = qk.tile([128, 2, T], BF16)   # [2heads x 64, batch, T]
        qT2 = qk.tile([128, 2, T], BF16)
        kT1 = qk.tile([128, 2, T], BF16)
        kT2 = qk.tile([128, 2, T], BF16)
        v5 = vpool.tile([128, NKB, 2, 2, 65], BF16)   # [Tmod, Tdiv, batch, head, dh+1]
        v5b = vpool.tile([128, NKB, 2, 2, 65], BF16)  # lam * v, ones col unscaled
        ones128 = const.tile([128, 1], BF16)
        nc.vector.memset(ones128[:], 1.0)
        for vv in (v5, v5b):
            nc.vector.tensor_copy(
                vv[:, :, :, :, 64:65],
                ones128[:, 0:1, None, None, None].to_broadcast([128, NKB, 2, 2, 1]),
            )
        a2a_in = [dram.tile([8, 64, TQ], BF16, name=f"a2ain{h}") for h in range(2)]
        a2a_out = [dram.tile([8, 64, TQ], BF16, name=f"a2aout{h}") for h in range(2)]

        # ---- phase 1: QKV projections, streaming xT chunks ----
        with tc.tile_pool(name="projw", bufs=1) as projw, \
             tc.tile_pool(name="xs", bufs=2) as xs, \
             tc.tile_pool(name="pproj", bufs=3, space="PSUM") as pproj:
            wq_sb = projw.tile([128, 8, 256], F32R)
            wk_sb = projw.tile([128, 8, 256], F32R)
            wv_sb = projw.tile([128, 8, 256], F32R)
            nc.gpsimd.dma_start(wq_sb[:], wqT3)
            nc.gpsimd.dma_start(wk_sb[:], wkT3)
            nc.gpsimd.dma_start(wv_sb[:], wvP3)
            for b2 in range(2):
                for tcn in range(NTC):
                    xc = xs.tile([128, 8, TC], F32R, tag="xc", name="xc")
                    nc.sync.dma_start(xc[:], xT3[b2][:, :, bass.ts(tcn, TC)])
                    for w_sb, dst1, dst2 in ((wq_sb, qT1, qT2), (wk_sb, kT1, kT2)):
                        for m in range(2):
                            ps = pproj.tile([128, TC], F32, tag="pqk", name="ps_qk")
                            for k in range(8):
                                nc.tensor.matmul(ps[:], w_sb[:, k, bass.ts(m, 128)],
                                                 xc[:, k, :], start=(k == 0), stop=(k == 7))
                            dst = dst1 if m == 0 else dst2
                            nc.vector.tensor_copy(dst[:, b2, bass.ts(tcn, TC)], ps[:])
                    for tt in range(TC // 128):
                        psv = pproj.tile([128, 256], F32, tag="pv", name="ps_v")
                        for k in range(8):
                            nc.tensor.matmul(psv[:], xc[:, k, bass.ts(tt, 128)],
                                             wv_sb[:, k, :], start=(k == 0), stop=(k == 7))
                        pv3 = psv[:, 0:128].rearrange("p (h d) -> p h d", h=2)
                        nc.vector.tensor_copy(v5[:, tcn * 4 + tt, b2, :, 0:64], pv3)
                        nc.vector.tensor_scalar_mul(
                            v5b[:, tcn * 4 + tt, b2, :, 0:64], pv3, lam128[:])

        # ---- phase 2: attention (tail weights prefetch meanwhile) ----
        wo_sb = tailw.tile([128, 8, C], F32R)
        wc_sb = tailw.tile([128, 8, 512], F32R)
        we_sb = tailw.tile([128, 4, C], F32R)
        nc.sync.dma_start(wo_sb[:], woT3)
        nc.sync.dma_start(wc_sb[:], wcT3)
        nc.sync.dma_start(we_sb[:], weT3)
        ypool = attn_ctx.enter_context(tc.tile_pool(name="ypool", bufs=1))
        yT = ypool.tile([128, 2, T], BF16)  # [2heads x 64, batch, T]

        with tc.tile_pool(name="epool", bufs=4) as epool, \
             tc.tile_pool(name="cmb", bufs=2) as cmb, \
             tc.tile_pool(name="psS", bufs=3, space="PSUM") as psS, \
             tc.tile_pool(name="psU", bufs=1, space="PSUM") as psU:
            for h2 in range(2):
                hb = 64 * h2
                for b2 in range(2):
                    for qt in range(NQT):
                        kmax = 4 * qt + 4
                        u1 = psU.tile([65, TQ], F32, tag="u1", name="u1")
                        u2 = psU.tile([65, TQ], F32, tag="u2", name="u2")
                        us = (u1, u2)

                        def emit_pv(p):
                            a, kb0, offs, e = p
                            v5x = v5 if a == 0 else v5b
                            u = us[a]
                            for i in (0, 1):
                                kb, qo = kb0 + i, offs[i]
                                nc.tensor.matmul(
                                    u[:, qo:], v5x[:, kb, b2, h2, :], e[:, i, qo:],
                                    start=(kb == 0), stop=(kb == kmax - 1),
                                    skip_group_check=True)

                        pend = []
                        for kb0 in range(0, kmax, 2):
                            for a, (qTa, kTa) in enumerate(((qT1, kT1), (qT2, kT2))):
                                st = psS.tile([128, 2, TQ], F32, tag="S", name="st")
                                e = epool.tile([128, 2, TQ], BF16, tag="E", name="et")
                                offs = []
                                for i in (0, 1):
                                    kb = kb0 + i
                                    qo = max(0, (kb - 4 * qt) * 128)
                                    offs.append(qo)
                                    nc.tensor.matmul(
                                        st[:, i, qo:],
                                        kTa[hb:hb + 64, b2, bass.ts(kb, KBS)],
                                        qTa[hb:hb + 64, b2, qt * TQ + qo:(qt + 1) * TQ],
                                        start=True, stop=True)
                                mo = min(offs)
                                nc.scalar.activation(e[:, :, mo:], st[:, :, mo:],
                                                     EXP, scale=SCALE)
                                for i in (0, 1):
                                    kb, qo = kb0 + i, offs[i]
                                    if kb >= 4 * qt:  # diagonal: staircase mask
                                        nc.vector.tensor_mul(e[:, i, qo:qo + 128],
                                                             e[:, i, qo:qo + 128],
                                                             mk_b[:])
                                pend.append((a, kb0, tuple(offs), e))
                                while len(pend) > 2:
                                    emit_pv(pend.pop(0))
                        for p in pend:
                            emit_pv(p)
                        # combine: yT[:, qt] = U1/Z1 - (lam*U2)/Z2
                        zr = cmb.tile([1, 2 * TQ], F32, tag="zr", name="zr")
                        nc.vector.tensor_copy(zr[:, 0:TQ], u1[64:65, :])
                        nc.vector.tensor_copy(zr[:, TQ:], u2[64:65, :])
                        rz = cmb.tile([1, 2 * TQ], F32, tag="rz", name="rz")
                        nc.vector.reciprocal_approx_fast(rz[:], zr[:])
                        rbs = cmb.tile([64, 2 * TQ], F32, tag="rbs", name="rbs")
                        nc.gpsimd.partition_broadcast(rbs[:], rz[:])
                        t1 = cmb.tile([64, TQ], F32, tag="t1", name="t1")
                        t2 = cmb.tile([64, TQ], F32, tag="t2", name="t2")
                        nc.vector.tensor_mul(t1[:], u1[0:64, :], rbs[:, 0:TQ])
                        nc.vector.tensor_mul(t2[:], u2[0:64, :], rbs[:, TQ:])
                        nc.vector.tensor_sub(yT[hb:hb + 64, b2, bass.ts(qt, TQ)],
                                             t1[:], t2[:])
                    # ship (h2, b2) yT rows into the A2A input shards
                    nc.sync.dma_start(
                        a2a_in[h2][b2 * 4:(b2 + 1) * 4, :, :].rearrange("j p t -> p j t"),
                        yT[hb:hb + 64, b2, :].rearrange("p (j t) -> p j t", t=TQ),
                    )
                nc.gpsimd.collective_compute(
                    "AllToAll", mybir.AluOpType.bypass,
                    replica_groups=[list(range(8))],
                    ins=[a2a_in[h2].opt()], outs=[a2a_out[h2].opt()],
                )

        # ---- phase 3: wo + compressor on the local (batch, T-quarter) ----
        attn_ctx.close()
        with tc.tile_pool(name="tail", bufs=1) as tail, \
             tc.tile_pool(name="opool", bufs=2) as opool, \
             tc.tile_pool(name="psT", bufs=2, space="PSUM") as psT:
            yb = tail.tile([128, 8, TQ], BF16)
            for kc in range(8):
                for h2 in range(2):
                    nc.sync.dma_start(yb[h2 * 64:(h2 + 1) * 64, kc, :],
                                      a2a_out[h2][kc, :, :])
            yf = tail.tile([128, 8, TQ], F32R)
            for kc in range(8):
                nc.vector.tensor_copy(yf[:, kc, :], yb[:, kc, :])
            zT = tail.tile([128, 8, TQ], F32R)
            for mt in range(8):
                pz = psT.tile([128, TQ], F32, tag="pz", name="pz")
                for kc in range(8):
                    nc.tensor.matmul(pz[:], wo_sb[:, kc, bass.ts(mt, 128)], yf[:, kc, :],
                                     start=(kc == 0), stop=(kc == 7))
                nc.vector.tensor_copy(zT[:, mt, :], pz[:])
            hT = tail.tile([128, 4, TQ], F32R)
            for mt in range(4):
                ph = psT.tile([128, TQ], F32, tag="ph", name="ph")
                for kc in range(8):
                    nc.tensor.matmul(ph[:], wc_sb[:, kc, bass.ts(mt, 128)], zT[:, kc, :],
                                     start=(kc == 0), stop=(kc == 7))
                nc.scalar.add(hT[:, mt, :], ph[:], bc_sb[:, mt:mt + 1])
            for tt in range(4):
                o = opool.tile([128, C], F32, tag="o", name="o")
                for half in range(2):
                    po = psT.tile([128, TQ], F32, tag="po", name="po")
                    for kc in range(4):
                        nc.tensor.matmul(po[:], hT[:, kc, bass.ts(tt, 128)],
                                         we_sb[:, kc, bass.ts(half, TQ)],
                                         start=(kc == 0), stop=(kc == 3))
                    nc.vector.tensor_add(o[:, bass.ts(half, TQ)], po[:],
                                         be_sb[:, bass.ts(half, TQ)])
                nc.sync.dma_start(out[bass.ts(tt, 128), :], o[:])


def _prep_inputs(inputs):
    g = {k: np.asarray(v, dtype=np.float32) for k, v in inputs.items()}
    x, wq, wk, wv, wo = g["x"], g["wq"], g["wk"], g["wv"], g["wo"]
    wc, bc, we, be = g["wc"], g["bc"], g["we"], g["be"]
    lv = np.concatenate([g["lq1"], g["lk1"], g["lq2"], g["lk2"]]).reshape(1, 64).astype(np.float32)
    mk = np.ascontiguousarray(np.tril(np.ones((128, 128), np.float32)).T)
    woT = np.ascontiguousarray(wo.T)
    wcT = np.ascontiguousarray(wc.T)
    weT = np.ascontiguousarray(we.T)

    bcT = np.ascontiguousarray(bc.reshape(4, 128).T)
    beR = np.ascontiguousarray(np.broadcast_to(be[None, :], (128, C)))
    xT0 = np.ascontiguousarray(x[0].T)
    xT1 = np.ascontiguousarray(x[1].T)
    in_maps = []
    for c in range(8):
        r0, r1 = c * 128, (c + 1) * 128
        wqs = np.ascontiguousarray(np.concatenate([wq[r0:r1], wq[C + r0:C + r1]], 0).T)
        wks = np.ascontiguousarray(np.concatenate([wk[r0:r1], wk[C + r0:C + r1]], 0).T)
        wvs = np.zeros((C, 256), dtype=np.float32)
        wvs[:, 0:128] = wv[r0:r1].T
        in_maps.append({
            "xT0": xT0, "xT1": xT1, "wqT": wqs, "wkT": wks,
            "wvP": np.ascontiguousarray(wvs), "woT": woT,
            "wcT": wcT, "weT": weT, "bcT": bcT, "beR": beR, "lv": lv, "mk": mk,
        })
    return in_maps


def _run(inputs, trace=False, trace_cores=None):
    if "nc" not in _CACHE:
        _CACHE["nc"] = _build()
    in_maps = _prep_inputs(inputs)
    r = run_bass_kernel_spmd(
        _CACHE["nc"], in_maps, core_ids=list(range(8)), trace=trace,
        trace_cores=trace_cores,
    )
    o = np.empty((B, T, C), dtype=np.float32)
    for c in range(8):
        b, hg = c // 4, c % 4
        o[b, hg * TQ:(hg + 1) * TQ, :] = r.results[c]["out"]
    return o, r


def kernel(**inputs) -> np.ndarray:
    o, _ = _run(inputs, trace=False)
    return o
